# revision 31
# baseline (speedup 1.0000x reference)
"""Trainium2 Bass kernel for ChannelMaxPool top-k masking.

Reference computation:
  x: (B=32, C=512, H=128, W=128) f32
  scores[b,c] = max |x[b,c,:,:]|
  top-128 channels by score (descending, jax.lax.top_k tie order)
  w[b,k] = exp(s_k - m) / sum_selected exp(s_j - m)
  y[b,k,:,:] = x[b, idx_k, :, :] * w[b,k]

Sharding: pure data-parallel, batch split across 8 NeuronCores
(4 samples per core), no communication.

Zero-gather scheme: the score pass streams x once in f32; the Scalar
engine down-converts every tile into a bf16 copy of the whole sample
kept in SBUF (16 MiB -> 128 KiB/partition, 5 rotating group slots).
Selected rows are scaled in place and written straight from SBUF to y
(bf16) with an indirect DMA whose per-row destination index is the
rank; unselected channels carry an out-of-bounds sentinel, which the
DMA silently skips.  Traffic/core: 128 MiB load + 16 MiB store =
144 MiB (~405 us HBM roofline), vs 192 MiB for a gather design.

Engine balance (measured rates): Scalar does the 16 bf16 converts
(~62 us/sample).  Score reduction is split: groups 2-3 absmax-reduce
on DVE; groups 0-1 square (into a small scratch, so the load tile is
released without waiting on GpSimd) + pool_max on the otherwise idle
GpSimd.  Ranking therefore runs on SQUARED scores (monotone and
rounding-exact: max_i fl(x_i^2) == fl((max_i |x_i|)^2)); sqrt is
applied only for the softmax weights.

Selection avoids the 40 us/sample serial top-8 chain entirely:
rank(c) = #{c': s'>s} + #{c'<c: s'==s} -- exactly jax.lax.top_k's
stable order -- computed with 16 [128x128] DVE compare blocks against
a row-replicated score vector, column-summed on the idle TensorEngine
(fp16 one-zero matmuls accumulate exactly in f32 PSUM).  This yields
rank/weight directly in channel layout: no gather, no transposes of
rank-ordered vectors.

Scatter dispatches ride the GpSimd queue; their emission is deferred
into the next sample's tile stream so a not-yet-ready dispatch never
head-of-line blocks the next sample's pool ops.
"""

import numpy as np

B, C, H, W = 32, 512, 128, 128
S = H * W
K = 128
N_CORES = 8
BL = B // N_CORES

S_TILE = 2048
NT = S // S_TILE  # tiles per channel group
CCH = C // 128  # channel groups
CSLOTS = 5
BIG = 1 << 20
FINE = 4  # sub-splits of the very last tile (scores finish earlier)


def _build_nc():
    import concourse.bass as bass
    import concourse.mybir as mybir
    from concourse import bacc
    from concourse.tile import TileContext

    f32 = mybir.dt.float32
    f16 = mybir.dt.float16
    bf16 = mybir.dt.bfloat16
    i32 = mybir.dt.int32

    from concourse.tile_rust import add_dep_helper

    nc = bacc.Bacc()
    x = nc.dram_tensor("x", [BL, C, S], f32, kind="ExternalInput")
    srow = nc.dram_tensor("srow", [BL, C], f32, kind="Internal")
    y = nc.dram_tensor("y", [BL, K, S], bf16, kind="ExternalOutput")
    y_rows = y[:].rearrange("b k s -> (b k) s")

    with TileContext(nc) as tc:
        with (
            tc.tile_pool(name="load", bufs=4) as load_pool,
            tc.tile_pool(name="cache", bufs=1) as cache_pool,
            tc.tile_pool(name="psum", bufs=2, space="PSUM") as psum_pool,
            tc.tile_pool(name="small", bufs=2) as small,
            tc.tile_pool(name="single", bufs=1) as single,
        ):
            # ---- constants ----
            ones_h = single.tile([K, 1], f16, tag="ones_h")
            nc.vector.memset(ones_h[:], 1.0)
            ones_f = single.tile([1, K], f32, tag="ones_f")
            nc.vector.memset(ones_f[:], 1.0)
            ones_c = single.tile([K, 1], f32, tag="ones_c")
            nc.vector.memset(ones_c[:], 1.0)
            rowio = single.tile([128, 128], f32, tag="rowio")
            nc.gpsimd.iota(
                rowio[:],
                pattern=[[1, 128]],
                base=0,
                channel_multiplier=0,
                allow_small_or_imprecise_dtypes=True,
            )
            colio = single.tile([128, 1], f32, tag="colio")
            nc.gpsimd.iota(
                colio[:],
                pattern=[[1, 1]],
                base=0,
                channel_multiplier=1,
                allow_small_or_imprecise_dtypes=True,
            )
            # strict lower-triangle (c' < c) tie mask for diagonal blocks
            ltmask = single.tile([128, 128], f16, tag="ltmask")
            nc.vector.tensor_tensor(
                out=ltmask[:],
                in0=rowio[:],
                in1=colio[:].to_broadcast([128, 128]),
                op=mybir.AluOpType.is_gt,
            )
            negb = single.tile([128, 1], f32, tag="negb")
            nc.vector.memset(negb[:], -4.0)
            cache = cache_pool.tile([128, CSLOTS * S], bf16, tag="cache")

            # deferred scale-chunk/scatter closures, spread through the
            # next sample's tile stream (one per flush point)
            pending = []

            for b in range(BL):
                # ---- pass 1: stream tiles; Scalar converts every tile to
                #      bf16 cache; scores via DVE absmax (groups 2-3) or
                #      gpsimd square+pool_max (groups 0-1) ----
                last_sample = b == BL - 1
                pdve = small.tile([128, CCH * NT + FINE - 1], f32, tag="pdve")
                tile_j = 0
                for g in range(CCH):
                    slot = (b * CCH + g) % CSLOTS
                    for t in range(NT):
                        if tile_j % 2 == 0 and pending:
                            pending.pop(0)()
                        tile_j += 1
                        last_tile = g == CCH - 1 and t == NT - 1
                        sub = FINE if last_tile else 1
                        sw = S_TILE // sub
                        for u in range(sub):
                            tile_in = load_pool.tile([128, S_TILE], f32, tag="ld")
                            s0 = t * S_TILE + u * sw
                            nc.sync.dma_start(
                                out=tile_in[:, :sw],
                                in_=x[b, g * 128 : (g + 1) * 128, s0 : s0 + sw],
                            )
                            nc.scalar.activation(
                                out=cache[:, slot * S + s0 : slot * S + s0 + sw],
                                in_=tile_in[:, :sw],
                                func=mybir.ActivationFunctionType.Copy,
                                bias=0.0,
                                scale=1.0,
                            )
                            col = g * NT + t + u
                            last_red = nc.vector.tensor_reduce(
                                out=pdve[:, col : col + 1],
                                in_=tile_in[:, :sw],
                                axis=mybir.AxisListType.X,
                                op=mybir.AluOpType.max,
                                apply_absolute_value=True,
                            )
                for p in pending:
                    p()
                pending = []
                # ---- assemble per-channel scores [128, CCH] ----
                scores_sq = small.tile([128, CCH], f32, tag="scores_sq")
                if True:
                    nc.vector.tensor_reduce(
                        out=scores_sq[:, : CCH - 1],
                        in_=pdve[:, : (CCH - 1) * NT].rearrange(
                            "p (g t) -> p g t", t=NT
                        ),
                        axis=mybir.AxisListType.X,
                        op=mybir.AluOpType.max,
                    )
                    nc.vector.tensor_reduce(
                        out=scores_sq[:, CCH - 1 : CCH],
                        in_=pdve[:, None, (CCH - 1) * NT : CCH * NT + FINE - 1],
                        axis=mybir.AxisListType.X,
                        op=mybir.AluOpType.max,
                    )
                # ---- replicate squared scores to all partitions ----
                # replicate scores to all partitions via one DRAM
                # round-trip: transpose-write [128,CCH] -> flat row, then a
                # broadcast read (every partition reads the same 2 KiB)
                srep = single.tile([128, C], f32, tag="srep")
                w_inst = nc.gpsimd.dma_start(
                    out=srow[b].rearrange("(g p) -> p g", p=128),
                    in_=scores_sq[:],
                )
                r_inst = nc.gpsimd.dma_start(
                    out=srep[:],
                    in_=srow[b : b + 1].rearrange("o c -> o c").to_broadcast(
                        [128, C]
                    ),
                )
                add_dep_helper(r_inst.ins, w_inst.ins, reason="srow RAW")
                # ---- rank(c) by counting: 16 compare blocks + TensorE
                #      column sums (exact: 0/1 fp16, f32 PSUM accum) ----
                rank4 = small.tile([128, CCH], f32, tag="rank4")
                comp = single.tile([128, C], f16, tag="comp")
                eqb = single.tile([128, 128], f16, tag="eqb")
                for g in range(CCH):
                    for gp in range(CCH):
                        cs = slice(gp * 128, (gp + 1) * 128)
                        in0 = scores_sq[:, gp : gp + 1].to_broadcast([128, 128])
                        in1 = srep[:, g * 128 : (g + 1) * 128]
                        nc.vector.tensor_tensor(
                            out=comp[:, cs], in0=in0, in1=in1,
                            op=mybir.AluOpType.is_ge
                            if gp < g
                            else mybir.AluOpType.is_gt,
                        )
                        if gp == g:
                            nc.vector.tensor_tensor(
                                out=eqb[:], in0=in0, in1=in1,
                                op=mybir.AluOpType.is_equal,
                            )
                            nc.vector.tensor_tensor(
                                out=eqb[:], in0=eqb[:], in1=ltmask[:],
                                op=mybir.AluOpType.mult,
                            )
                            nc.vector.tensor_tensor(
                                out=comp[:, cs], in0=comp[:, cs], in1=eqb[:],
                                op=mybir.AluOpType.add,
                            )
                    ps = psum_pool.tile([128, 1], f32, tag="ps_rank")
                    for gp in range(CCH):
                        nc.tensor.matmul(
                            ps[:],
                            comp[:, gp * 128 : (gp + 1) * 128],
                            ones_h[:],
                            start=gp == 0,
                            stop=gp == CCH - 1,
                        )
                    nc.vector.tensor_copy(rank4[:, g : g + 1], ps[:])
                # ---- softmax weights over the selected set ----
                e4 = small.tile([128, CCH], f32, tag="e4")
                nc.scalar.activation(
                    out=e4[:], in_=scores_sq[:],
                    func=mybir.ActivationFunctionType.Exp,
                    bias=negb[:, 0:1], scale=1.0,
                )
                selm = small.tile([128, CCH], f32, tag="selm")
                nc.vector.tensor_scalar(
                    out=selm[:], in0=rank4[:],
                    scalar1=float(K) - 0.5, scalar2=None,
                    op0=mybir.AluOpType.is_lt,
                )
                me4 = small.tile([128, CCH], f32, tag="me4")
                nc.vector.tensor_tensor(
                    out=me4[:], in0=e4[:], in1=selm[:], op=mybir.AluOpType.mult
                )
                ps_d = psum_pool.tile([1, CCH], f32, tag="ps_d")
                nc.tensor.matmul(ps_d[:], ones_c[:], me4[:])
                dn4 = small.tile([1, CCH], f32, tag="dn4")
                nc.vector.tensor_copy(dn4[:], ps_d[:])
                denom = small.tile([1, 1], f32, tag="denom")
                nc.vector.reduce_sum(
                    out=denom[:], in_=dn4[:], axis=mybir.AxisListType.X
                )
                sinv = small.tile([1, 1], f32, tag="sinv")
                nc.vector.reciprocal(sinv[:], denom[:])
                ps_s = psum_pool.tile([128, 1], f32, tag="ps_sinv")
                nc.tensor.matmul(ps_s[:], ones_f[:], sinv[:])
                sinvb = small.tile([128, 1], f32, tag="sinvb")
                nc.vector.tensor_copy(sinvb[:], ps_s[:])
                w4 = small.tile([128, CCH], f32, tag="w4")
                nc.vector.tensor_scalar_mul(w4[:], e4[:], sinvb[:, 0:1])
                # ---- scatter row index: rank + b*K, OOB if unselected ----
                nb = small.tile([128, CCH], f32, tag="nb")
                nc.vector.tensor_scalar(
                    out=nb[:], in0=rank4[:],
                    scalar1=float(K) - 0.5, scalar2=float(BIG),
                    op0=mybir.AluOpType.is_gt, op1=mybir.AluOpType.mult,
                )
                off_f = small.tile([128, CCH], f32, tag="off_f")
                nc.vector.scalar_tensor_tensor(
                    out=off_f[:], in0=rank4[:], scalar=float(b * K), in1=nb[:],
                    op0=mybir.AluOpType.add, op1=mybir.AluOpType.add,
                )
                off_i = small.tile([128, CCH], i32, tag="off_i")
                nc.vector.tensor_copy(off_i[:], off_f[:])
                # ---- scale cached rows in place (DVE bf16, ~4.3us) and
                #      scatter straight to y.  Group 0 goes out inline;
                #      groups 1-3 are deferred into the next sample's tile
                #      stream so the epilogue burst on the in-order DVE
                #      queue does not starve the next sample's reduces ----
                CH = S // 4

                def chunk_fn(g, q, eng, b=b, w4=w4):
                    def fn():
                        slot = (b * CCH + g) % CSLOTS
                        cg = cache[:, slot * S + q * CH : slot * S + (q + 1) * CH]
                        if eng == "v":
                            i = nc.vector.tensor_scalar_mul(
                                cg, cg, w4[:, g : g + 1]
                            )
                        else:
                            i = nc.scalar.activation(
                                out=cg, in_=cg,
                                func=mybir.ActivationFunctionType.Copy,
                                bias=0.0, scale=w4[:, g : g + 1],
                            )
                        return [i]
                    return fn

                def sc_fn(g, b=b, off_i=off_i):
                    def fn():
                        slot = (b * CCH + g) % CSLOTS
                        return [] if nc.gpsimd.indirect_dma_start(
                            out=y_rows,
                            out_offset=bass.IndirectOffsetOnAxis(
                                ap=off_i[:, g : g + 1], axis=0
                            ),
                            in_=cache[:, slot * S : (slot + 1) * S],
                            in_offset=None,
                            bounds_check=BL * K - 1,
                            oob_is_err=False,
                        ) else []
                    return fn

                if last_sample:
                    # tail: run DVE and Scalar chunk-scales in parallel
                    for g in range(CCH):
                        for q in range(4):
                            chunk_fn(g, q, "v" if q % 2 == 0 else "s")()
                        sc_fn(g)()
                else:
                    # group 0 inline (its scatter gates the next sample's
                    # group-1 converts); groups 1-3 deferred
                    for q in range(4):
                        chunk_fn(0, q, "v")()
                    sc_fn(0)()
                    pending = []
                    for g in range(1, CCH):
                        pending += [chunk_fn(g, q, "v") for q in range(4)]
                        pending.append(sc_fn(g))
    if not nc.is_finalized():
        nc.finalize()
    return nc


_NC_CACHE = None


def _get_nc():
    global _NC_CACHE
    if _NC_CACHE is None:
        _NC_CACHE = _build_nc()
    return _NC_CACHE


def _run(x, trace=False):
    from concourse.bass_utils import run_bass_kernel_spmd

    nc = _get_nc()
    xr = np.ascontiguousarray(x, dtype=np.float32).reshape(N_CORES, BL, C, S)
    in_maps = [{"x": xr[c]} for c in range(N_CORES)]
    res = run_bass_kernel_spmd(nc, in_maps, list(range(N_CORES)), trace=trace)
    out = np.empty((B, K, H, W), dtype=np.float32)
    for c in range(N_CORES):
        out[c * BL : (c + 1) * BL] = (
            np.asarray(res.results[c]["y"]).astype(np.float32).reshape(BL, K, H, W)
        )
    return out, res


def kernel(x):
    out, _ = _run(x, trace=False)
    return out


# revision 32
# speedup vs baseline: 1.1895x; 1.1895x over previous
"""Trainium2 Bass kernel for ChannelMaxPool top-k masking.

Reference computation:
  x: (B=32, C=512, H=128, W=128) f32
  scores[b,c] = max |x[b,c,:,:]|
  top-128 channels by score (descending, jax.lax.top_k tie order)
  w[b,k] = exp(s_k - m) / sum_selected exp(s_j - m)
  y[b,k,:,:] = x[b, idx_k, :, :] * w[b,k]

Sharding: pure data-parallel, batch split across 8 NeuronCores
(4 samples per core), no communication.

Zero-gather scheme: the score pass streams x once in f32 (2048-wide
tiles, 4 load buffers); while DVE absmax-reduces exact scores, the
Scalar engine down-converts every tile into a bf16 copy of the whole
sample kept in SBUF (16 MiB -> 128 KiB/partition, 5 rotating group
slots so a sample's converts only ever wait on scatters from more
than one sample ago).  Selected rows are scaled in place and written
straight from SBUF to y (bf16) with an indirect DMA whose per-row
destination index is the rank; unselected channels carry an
out-of-bounds sentinel, which the DMA silently skips (zero traffic).
y is bf16 (~0.5% max element error, inside the 2e-2 gate) and
upconverted on the host.  Traffic/core: 128 MiB load + 16 MiB store
= 144 MiB (~405 us HBM roofline) vs 192 MiB for a gather design.

Selection avoids any serial top-k chain: rank(c) = #{c': s'>s} +
#{c'<c: s'==s} -- exactly jax.lax.top_k's stable order -- computed
with 16 [128x128] DVE compare blocks (is_gt / is_ge, plus an
eq*lower-triangle term on diagonal blocks) against a row-replicated
score vector, column-summed on the idle TensorEngine (one-zero fp16
matmuls accumulate exactly in f32 PSUM).  This yields rank and
weight directly in channel layout (~11 us/sample vs ~40 us for the
top-8-extraction chain).  Score replication uses one DRAM round-trip
(transpose-write + stride-0 broadcast read) instead of a 7-step
SBUF doubling chain whose serial semaphore latency (~40 us) starved
the next sample's loads.  The softmax denominator is summed with a
TensorE ones-matmul; exp uses a constant shift (no exact max needed;
the shift cancels in the normalized ratio).

The scale (by w, on DVE at bf16 2x rate) is emitted in 4-KiB-column
chunks, group 0 inline and groups 1-3 deferred into the next
sample's tile stream (one closure per two tiles), so the epilogue
burst on the in-order DVE queue does not starve the next sample's
reduces; deferral never crosses a cache-slot conflict (closure k
flushes before the tile whose convert would invert the scatter's
read-after-write).  Epilogue small DMAs dispatch from the GpSimd
queue, keeping the sync queue pure loads.  The last sample's chunks
alternate DVE/Scalar to halve the exposed tail.

Measured: ~545-650 ns/core run-to-run (device-noisy), vs 632 us for
the gather-based baseline; 144 MiB/core HBM roofline is ~405 us.
"""

import numpy as np

B, C, H, W = 32, 512, 128, 128
S = H * W
K = 128
N_CORES = 8
BL = B // N_CORES

S_TILE = 2048
NT = S // S_TILE  # tiles per channel group
CCH = C // 128  # channel groups
CSLOTS = 5
BIG = 1 << 20
FINE = 4  # sub-splits of the very last tile (scores finish earlier)


def _build_nc():
    import concourse.bass as bass
    import concourse.mybir as mybir
    from concourse import bacc
    from concourse.tile import TileContext

    f32 = mybir.dt.float32
    f16 = mybir.dt.float16
    bf16 = mybir.dt.bfloat16
    i32 = mybir.dt.int32

    from concourse.tile_rust import add_dep_helper

    nc = bacc.Bacc()
    x = nc.dram_tensor("x", [BL, C, S], f32, kind="ExternalInput")
    srow = nc.dram_tensor("srow", [BL, C], f32, kind="Internal")
    y = nc.dram_tensor("y", [BL, K, S], bf16, kind="ExternalOutput")
    y_rows = y[:].rearrange("b k s -> (b k) s")

    with TileContext(nc) as tc:
        with (
            tc.tile_pool(name="load", bufs=4) as load_pool,
            tc.tile_pool(name="cache", bufs=1) as cache_pool,
            tc.tile_pool(name="psum", bufs=2, space="PSUM") as psum_pool,
            tc.tile_pool(name="small", bufs=2) as small,
            tc.tile_pool(name="single", bufs=1) as single,
        ):
            # ---- constants ----
            ones_h = single.tile([K, 1], f16, tag="ones_h")
            nc.vector.memset(ones_h[:], 1.0)
            ones_f = single.tile([1, K], f32, tag="ones_f")
            nc.vector.memset(ones_f[:], 1.0)
            ones_c = single.tile([K, 1], f32, tag="ones_c")
            nc.vector.memset(ones_c[:], 1.0)
            rowio = single.tile([128, 128], f32, tag="rowio")
            nc.gpsimd.iota(
                rowio[:],
                pattern=[[1, 128]],
                base=0,
                channel_multiplier=0,
                allow_small_or_imprecise_dtypes=True,
            )
            colio = single.tile([128, 1], f32, tag="colio")
            nc.gpsimd.iota(
                colio[:],
                pattern=[[1, 1]],
                base=0,
                channel_multiplier=1,
                allow_small_or_imprecise_dtypes=True,
            )
            # strict lower-triangle (c' < c) tie mask for diagonal blocks
            ltmask = single.tile([128, 128], f16, tag="ltmask")
            nc.vector.tensor_tensor(
                out=ltmask[:],
                in0=rowio[:],
                in1=colio[:].to_broadcast([128, 128]),
                op=mybir.AluOpType.is_gt,
            )
            negb = single.tile([128, 1], f32, tag="negb")
            nc.vector.memset(negb[:], -4.0)
            cache = cache_pool.tile([128, CSLOTS * S], bf16, tag="cache")

            # deferred scale-chunk/scatter closures, spread through the
            # next sample's tile stream (one per flush point)
            pending = []

            for b in range(BL):
                # ---- pass 1: stream tiles; Scalar converts every tile to
                #      bf16 cache; scores via DVE absmax (groups 2-3) or
                #      gpsimd square+pool_max (groups 0-1) ----
                last_sample = b == BL - 1
                pdve = small.tile([128, CCH * NT + FINE - 1], f32, tag="pdve")
                tile_j = 0
                for g in range(CCH):
                    slot = (b * CCH + g) % CSLOTS
                    for t in range(NT):
                        if tile_j % 2 == 0 and pending:
                            pending.pop(0)()
                        tile_j += 1
                        last_tile = g == CCH - 1 and t == NT - 1
                        sub = FINE if last_tile else 1
                        sw = S_TILE // sub
                        for u in range(sub):
                            tile_in = load_pool.tile([128, S_TILE], f32, tag="ld")
                            s0 = t * S_TILE + u * sw
                            nc.sync.dma_start(
                                out=tile_in[:, :sw],
                                in_=x[b, g * 128 : (g + 1) * 128, s0 : s0 + sw],
                            )
                            nc.scalar.activation(
                                out=cache[:, slot * S + s0 : slot * S + s0 + sw],
                                in_=tile_in[:, :sw],
                                func=mybir.ActivationFunctionType.Copy,
                                bias=0.0,
                                scale=1.0,
                            )
                            col = g * NT + t + u
                            last_red = nc.vector.tensor_reduce(
                                out=pdve[:, col : col + 1],
                                in_=tile_in[:, :sw],
                                axis=mybir.AxisListType.X,
                                op=mybir.AluOpType.max,
                                apply_absolute_value=True,
                            )
                for p in pending:
                    p()
                pending = []
                # ---- assemble per-channel scores [128, CCH] ----
                scores_sq = small.tile([128, CCH], f32, tag="scores_sq")
                nc.vector.tensor_reduce(
                    out=scores_sq[:, : CCH - 1],
                    in_=pdve[:, : (CCH - 1) * NT].rearrange(
                        "p (g t) -> p g t", t=NT
                    ),
                    axis=mybir.AxisListType.X,
                    op=mybir.AluOpType.max,
                )
                nc.vector.tensor_reduce(
                    out=scores_sq[:, CCH - 1 : CCH],
                    in_=pdve[:, None, (CCH - 1) * NT : CCH * NT + FINE - 1],
                    axis=mybir.AxisListType.X,
                    op=mybir.AluOpType.max,
                )
                # ---- replicate squared scores to all partitions ----
                # replicate scores to all partitions via one DRAM
                # round-trip: transpose-write [128,CCH] -> flat row, then a
                # broadcast read (every partition reads the same 2 KiB)
                srep = single.tile([128, C], f32, tag="srep")
                w_inst = nc.gpsimd.dma_start(
                    out=srow[b].rearrange("(g p) -> p g", p=128),
                    in_=scores_sq[:],
                )
                r_inst = nc.gpsimd.dma_start(
                    out=srep[:],
                    in_=srow[b : b + 1].rearrange("o c -> o c").to_broadcast(
                        [128, C]
                    ),
                )
                add_dep_helper(r_inst.ins, w_inst.ins, reason="srow RAW")
                # ---- rank(c) by counting: 16 compare blocks + TensorE
                #      column sums (exact: 0/1 fp16, f32 PSUM accum) ----
                rank4 = small.tile([128, CCH], f32, tag="rank4")
                comp = single.tile([128, C], f16, tag="comp")
                eqb = single.tile([128, 128], f16, tag="eqb")
                for g in range(CCH):
                    for gp in range(CCH):
                        cs = slice(gp * 128, (gp + 1) * 128)
                        in0 = scores_sq[:, gp : gp + 1].to_broadcast([128, 128])
                        in1 = srep[:, g * 128 : (g + 1) * 128]
                        nc.vector.tensor_tensor(
                            out=comp[:, cs], in0=in0, in1=in1,
                            op=mybir.AluOpType.is_ge
                            if gp < g
                            else mybir.AluOpType.is_gt,
                        )
                        if gp == g:
                            nc.vector.tensor_tensor(
                                out=eqb[:], in0=in0, in1=in1,
                                op=mybir.AluOpType.is_equal,
                            )
                            nc.vector.tensor_tensor(
                                out=eqb[:], in0=eqb[:], in1=ltmask[:],
                                op=mybir.AluOpType.mult,
                            )
                            nc.vector.tensor_tensor(
                                out=comp[:, cs], in0=comp[:, cs], in1=eqb[:],
                                op=mybir.AluOpType.add,
                            )
                    ps = psum_pool.tile([128, 1], f32, tag="ps_rank")
                    for gp in range(CCH):
                        nc.tensor.matmul(
                            ps[:],
                            comp[:, gp * 128 : (gp + 1) * 128],
                            ones_h[:],
                            start=gp == 0,
                            stop=gp == CCH - 1,
                        )
                    nc.vector.tensor_copy(rank4[:, g : g + 1], ps[:])
                # ---- softmax weights over the selected set ----
                e4 = small.tile([128, CCH], f32, tag="e4")
                nc.scalar.activation(
                    out=e4[:], in_=scores_sq[:],
                    func=mybir.ActivationFunctionType.Exp,
                    bias=negb[:, 0:1], scale=1.0,
                )
                selm = small.tile([128, CCH], f32, tag="selm")
                nc.vector.tensor_scalar(
                    out=selm[:], in0=rank4[:],
                    scalar1=float(K) - 0.5, scalar2=None,
                    op0=mybir.AluOpType.is_lt,
                )
                me4 = small.tile([128, CCH], f32, tag="me4")
                nc.vector.tensor_tensor(
                    out=me4[:], in0=e4[:], in1=selm[:], op=mybir.AluOpType.mult
                )
                ps_d = psum_pool.tile([1, CCH], f32, tag="ps_d")
                nc.tensor.matmul(ps_d[:], ones_c[:], me4[:])
                dn4 = small.tile([1, CCH], f32, tag="dn4")
                nc.vector.tensor_copy(dn4[:], ps_d[:])
                denom = small.tile([1, 1], f32, tag="denom")
                nc.vector.reduce_sum(
                    out=denom[:], in_=dn4[:], axis=mybir.AxisListType.X
                )
                sinv = small.tile([1, 1], f32, tag="sinv")
                nc.vector.reciprocal(sinv[:], denom[:])
                ps_s = psum_pool.tile([128, 1], f32, tag="ps_sinv")
                nc.tensor.matmul(ps_s[:], ones_f[:], sinv[:])
                sinvb = small.tile([128, 1], f32, tag="sinvb")
                nc.vector.tensor_copy(sinvb[:], ps_s[:])
                w4 = small.tile([128, CCH], f32, tag="w4")
                nc.vector.tensor_scalar_mul(w4[:], e4[:], sinvb[:, 0:1])
                # ---- scatter row index: rank + b*K, OOB if unselected ----
                nb = small.tile([128, CCH], f32, tag="nb")
                nc.vector.tensor_scalar(
                    out=nb[:], in0=rank4[:],
                    scalar1=float(K) - 0.5, scalar2=float(BIG),
                    op0=mybir.AluOpType.is_gt, op1=mybir.AluOpType.mult,
                )
                off_f = small.tile([128, CCH], f32, tag="off_f")
                nc.vector.scalar_tensor_tensor(
                    out=off_f[:], in0=rank4[:], scalar=float(b * K), in1=nb[:],
                    op0=mybir.AluOpType.add, op1=mybir.AluOpType.add,
                )
                off_i = small.tile([128, CCH], i32, tag="off_i")
                nc.vector.tensor_copy(off_i[:], off_f[:])
                # ---- scale cached rows in place (DVE bf16, ~4.3us) and
                #      scatter straight to y.  Group 0 goes out inline;
                #      groups 1-3 are deferred into the next sample's tile
                #      stream so the epilogue burst on the in-order DVE
                #      queue does not starve the next sample's reduces ----
                CH = S // 4

                def chunk_fn(g, q, eng, b=b, w4=w4):
                    def fn():
                        slot = (b * CCH + g) % CSLOTS
                        cg = cache[:, slot * S + q * CH : slot * S + (q + 1) * CH]
                        if eng == "v":
                            i = nc.vector.tensor_scalar_mul(
                                cg, cg, w4[:, g : g + 1]
                            )
                        else:
                            i = nc.scalar.activation(
                                out=cg, in_=cg,
                                func=mybir.ActivationFunctionType.Copy,
                                bias=0.0, scale=w4[:, g : g + 1],
                            )
                        return [i]
                    return fn

                def sc_fn(g, b=b, off_i=off_i):
                    def fn():
                        slot = (b * CCH + g) % CSLOTS
                        return [] if nc.gpsimd.indirect_dma_start(
                            out=y_rows,
                            out_offset=bass.IndirectOffsetOnAxis(
                                ap=off_i[:, g : g + 1], axis=0
                            ),
                            in_=cache[:, slot * S : (slot + 1) * S],
                            in_offset=None,
                            bounds_check=BL * K - 1,
                            oob_is_err=False,
                        ) else []
                    return fn

                if last_sample:
                    # tail: run DVE and Scalar chunk-scales in parallel
                    for g in range(CCH):
                        for q in range(4):
                            chunk_fn(g, q, "v" if q % 2 == 0 else "s")()
                        sc_fn(g)()
                else:
                    # group 0 inline (its scatter gates the next sample's
                    # group-1 converts); groups 1-3 deferred
                    for q in range(4):
                        chunk_fn(0, q, "v")()
                    sc_fn(0)()
                    pending = []
                    for g in range(1, CCH):
                        pending += [chunk_fn(g, q, "v") for q in range(4)]
                        pending.append(sc_fn(g))
    if not nc.is_finalized():
        nc.finalize()
    return nc


_NC_CACHE = None


def _get_nc():
    global _NC_CACHE
    if _NC_CACHE is None:
        _NC_CACHE = _build_nc()
    return _NC_CACHE


def _run(x, trace=False):
    from concourse.bass_utils import run_bass_kernel_spmd

    nc = _get_nc()
    xr = np.ascontiguousarray(x, dtype=np.float32).reshape(N_CORES, BL, C, S)
    in_maps = [{"x": xr[c]} for c in range(N_CORES)]
    res = run_bass_kernel_spmd(nc, in_maps, list(range(N_CORES)), trace=trace)
    out = np.empty((B, K, H, W), dtype=np.float32)
    for c in range(N_CORES):
        out[c * BL : (c + 1) * BL] = (
            np.asarray(res.results[c]["y"]).astype(np.float32).reshape(BL, K, H, W)
        )
    return out, res


def kernel(x):
    out, _ = _run(x, trace=False)
    return out
